# revision 5
# baseline (speedup 1.0000x reference)
"""Trainium2 Bass kernel for nn_MultiHeadAttention (x:[2,2048,512], 8 heads, d=64).

Sharding: 8 cores = 2 batches x 4 head-pairs. Each core computes the QKV
projection for its 2 heads, the attention, and a partial (row-split) O
projection. Host sums the 4 partials per batch and adds the output bias.

All matmul inputs are bf16 (validated ~6e-3 max rel err vs the 2e-2 gate);
accumulation is always fp32 in PSUM. Host pre-arranges every input so each
DMA descriptor row is contiguous. Partial outputs return as bf16.

Per-core schedule (single Act-engine EXP stream is the bound, ~69us):
  t0      DMA issues: xT k0 + weights on sync; k1..k3 on gpsimd, each
          preceded by a guard that READS the previous k-tile and WRITES
          into the next tile's region (a real WAW dep the scheduler must
          honor), serializing transfers so k0 lands first and the k-major
          projection pipelines with arrival; PE warm-up fillers
  phase A k-major KQ projection accumulating in psum per arriving k-tile:
          KT qt0-3 (ps pool banks) + QT qt0/qt1 (av banks); k3 row emits
          KT0/QT0 first and their casts gate scores(0) only
  V proj  bf16: 4 seq-blocks share one PSUM bank (sequential per-region
          accumulation groups on the in-order PE); group 0 up front,
          groups 1-3 threaded one block per chunk
  chunks  64 chunks (qt,ks), per-chunk PE order: scores(n+1) FIRST, then
          one piece of deferred work, then AV(n). EXP on Act (psum fp32
          -> sbuf bf16, scale fused); AV with M=65 ones-column trick.
          AV banks ALTERNATE per qt parity (a0/a1 <-> po/tr) so the next
          qt's AV opens banks freed chunks earlier - no head-of-line
          stall on the in-order PE queue at qt boundaries.
  norm    per qt on the off-duty bank pair's schedule: avc+s_row staged
          (frees av banks), reciprocal, gpsimd partition broadcast, DVE
          multiply -> Z; O-projection of qt-1 threaded into qt's chunks
          on the off-duty banks; one batched out-DMA per qt; tail keeps
          the PE warm with ps-pool fillers through the final normalize
"""

import sys

import numpy as np

for _p in ("/opt/trn_rl_repo",):
    if _p not in sys.path:
        sys.path.insert(0, _p)

import ml_dtypes  # noqa: E402

import concourse.bass as bass  # noqa: E402
import concourse.tile as tile  # noqa: E402
from concourse import bacc, mybir  # noqa: E402
from concourse.bass_utils import run_bass_kernel_spmd  # noqa: E402

EMBED = 512
NH = 8
HD = 64
S = 2048
B = 2
SCALE = HD ** -0.5
F32 = mybir.dt.float32
F32R = mybir.dt.float32r
BF16 = mybir.dt.bfloat16

N_KT = EMBED // 128   # 4 contraction k-tiles for the projections
N_QT = S // 512       # 4 q column tiles
N_ST = S // 128       # 16 seq tiles of 128

N_FILL = 8            # PE warm-up matmuls during the DMA phase
N_TAIL_FILL = 14      # PE keep-warm matmuls through the tail normalize


def build_nc():
    nc = bacc.Bacc("TRN2", target_bir_lowering=False, debug=False)

    xTb_d = nc.dram_tensor("xTb", [128, N_KT, S], BF16, kind="ExternalInput").ap()
    wqb_d = nc.dram_tensor("wqb", [128, N_KT, 128], BF16, kind="ExternalInput").ap()
    wkb_d = nc.dram_tensor("wkb", [128, N_KT, 128], BF16, kind="ExternalInput").ap()
    wvb_d = nc.dram_tensor("wvb", [128, N_KT, 128], BF16, kind="ExternalInput").ap()
    wo_d = nc.dram_tensor("wo", [128, EMBED], F32R, kind="ExternalInput").ap()
    out_d = nc.dram_tensor("out", [S, EMBED], BF16, kind="ExternalOutput").ap()
    out_r = out_d.rearrange("(m p) e -> p m e", p=128)

    with tile.TileContext(nc) as tc:
        with (
            tc.tile_pool(name="persist", bufs=1) as persist,
            tc.tile_pool(name="pt_pool", bufs=6) as pt_pool,
            tc.tile_pool(name="norm", bufs=2) as norm_pool,
            tc.tile_pool(name="ostage", bufs=2) as ostage,
            tc.tile_pool(name="ps", bufs=2, space="PSUM") as ps_pool,
            tc.tile_pool(name="ps_a0", bufs=1, space="PSUM") as a0_pool,
            tc.tile_pool(name="ps_a1", bufs=1, space="PSUM") as a1_pool,
            tc.tile_pool(name="ps_po", bufs=1, space="PSUM") as po_pool,
            tc.tile_pool(name="ps_tr", bufs=1, space="PSUM") as tr_pool,
        ):
            # ---- DMA issues first. sync: xT k0, then small weights.
            # gpsimd: k1..k3, serialized by guard ops (read k-1 tile, write
            # 2 elements into k's region -> WAW dep on the k DMA). ----
            wkb_sb = persist.tile([128, N_KT, 128], BF16)
            wqb_sb = persist.tile([128, N_KT, 128], BF16)
            wvb_sb = persist.tile([128, N_KT, 128], BF16)
            wo_sb = persist.tile([128, EMBED], F32R)
            xTb_sb = persist.tile([128, N_KT, S], BF16)
            nc.sync.dma_start(out=xTb_sb[:, 0, :], in_=xTb_d[:, 0, :])
            nc.sync.dma_start(out=wkb_sb, in_=wkb_d)
            nc.sync.dma_start(out=wqb_sb, in_=wqb_d)
            nc.sync.dma_start(out=wvb_sb, in_=wvb_d)
            nc.sync.dma_start(out=wo_sb, in_=wo_d)

            dummy = persist.tile([128, 512], BF16)
            nc.gpsimd.memset(dummy, 0.0)
            for k in range(1, N_KT):
                nc.gpsimd.tensor_scalar_add(
                    out=xTb_sb[0:1, k, 0:2], in0=xTb_sb[0:1, k - 1, 0:2], scalar1=0.0,
                )
                nc.gpsimd.dma_start(out=xTb_sb[:, k, :], in_=xTb_d[:, k, :])
            # V with a baked all-ones 65th column: the M=65 AV matmuls then
            # produce the softmax denominators in psum row 64 for free
            V_sb = persist.tile([128, N_ST, 2, HD + 1], BF16)
            nc.gpsimd.memset(V_sb, 1.0)

            # ---- PE warm-up fillers (p-state ramp through the DMA wait) ----
            fill_pools = (a0_pool, a1_pool, po_pool, tr_pool)
            for i in range(N_FILL):
                fl = fill_pools[i % 4].tile([128, 512], F32, tag="b", name="fl")
                nc.tensor.matmul(fl, dummy[:, 0:128], dummy, start=True, stop=True)

            # ---- phase A: k-major KQ projection. The last k row emits
            # KT qt0 / QT qt0 first so their casts gate scores(0) only ----
            KTps = [ps_pool.tile([128, 2, 512], F32, tag="ps", name="KTps") for _ in range(2)]
            QT0ps = a0_pool.tile([128, 512], F32, tag="b", name="QT0ps")
            QT1ps = a1_pool.tile([128, 512], F32, tag="b", name="QT1ps")

            def kq_mm(j, k):
                st, sp = k == 0, k == N_KT - 1
                if j < 4:
                    nc.tensor.matmul(
                        KTps[j // 2][:, j % 2, :],
                        wkb_sb[:, k, :], xTb_sb[:, k, bass.ts(j, 512)],
                        start=st, stop=sp,
                    )
                else:
                    nc.tensor.matmul(
                        QT0ps if j == 4 else QT1ps, wqb_sb[:, k, :],
                        xTb_sb[:, k, bass.ts(j - 4, 512)], start=st, stop=sp,
                    )

            for k in range(N_KT - 1):
                for j in range(6):
                    kq_mm(j, k)
            for j in (0, 4, 1, 5, 2, 3):  # KT0, QT0 first on the last k row
                kq_mm(j, N_KT - 1)
            KTb = persist.tile([128, S], BF16)
            QTb = persist.tile([128, S], BF16)
            nc.vector.tensor_copy(KTb[:, 0:512], KTps[0][:, 0, :])
            nc.vector.tensor_copy(QTb[:, 0:512], QT0ps)
            nc.vector.tensor_copy(QTb[:, 512:1024], QT1ps)
            nc.vector.tensor_copy(KTb[:, 512:1024], KTps[0][:, 1, :])
            nc.vector.tensor_copy(KTb[:, 1024:1536], KTps[1][:, 0, :])
            nc.vector.tensor_copy(KTb[:, 1536:2048], KTps[1][:, 1, :])

            # ---- V projection: 4 seq-blocks [128,128] side by side in one
            # psum bank; per-region accumulation groups run sequentially on
            # the in-order PE so plain start/stop per block is safe ----
            vg_state = {}

            def emit_vblock(pool, g, b):
                if b == 0:
                    vg_state[g] = pool.tile([128, 512], F32, tag="b", name="vg")
                vg = vg_state[g]
                stq = 4 * g + b
                for k in range(N_KT):
                    nc.tensor.matmul(
                        vg[:, bass.ts(b, 128)],
                        xTb_sb[:, k, bass.ts(stq, 128)], wvb_sb[:, k, :],
                        start=(k == 0), stop=(k == N_KT - 1),
                    )
                nc.vector.tensor_copy(
                    V_sb[:, stq, :, 0:HD],
                    vg.rearrange("p (b h d) -> p b h d", b=4, h=2)[:, b],
                )

            # ---- attention chunk stream ----
            Z_sb = persist.tile([128, S], F32R)  # normalized attn out^T, 2 heads
            ost_state = {}

            def emit_scores(n):
                qt, ks = n // N_ST, n % N_ST
                s = ps_pool.tile([128, 2, 512], F32, tag="ps", name="s")
                kk = bass.ts(ks, 128)
                qs = bass.ts(qt, 512)
                nc.tensor.matmul(
                    s[:, 0, :], KTb[0:64, kk], QTb[0:64, qs], start=True, stop=True,
                )
                nc.tensor.matmul(
                    s[:, 1, :], KTb[64:128, kk], QTb[64:128, qs], start=True, stop=True,
                )
                return s

            def emit_oproj_piece(src_qt, mi, pool):
                # one N=512 m-block of the O-projection for source tile
                # src_qt; 4 pieces land in one ostage tile, one DMA per qt
                m = 4 * src_qt + mi
                if mi == 0:
                    ost_state[src_qt] = ostage.tile([128, 4, 512], BF16, tag="ot", name="ost")
                po = pool.tile([128, 512], F32, tag="b", name="po")
                nc.tensor.matmul(
                    po, Z_sb[:, bass.ts(m, 128)], wo_sb, start=True, stop=True,
                )
                nc.vector.tensor_copy(ost_state[src_qt][:, mi, :], po)
                if mi == 3:
                    nc.sync.dma_start(
                        out=out_r[:, 4 * src_qt:4 * src_qt + 4, :],
                        in_=ost_state[src_qt],
                    )

            def emit_qtproj(qx, pool):
                qp = pool.tile([128, 512], F32, tag="b", name="qp")
                for k in range(N_KT):
                    nc.tensor.matmul(
                        qp, wqb_sb[:, k, :], xTb_sb[:, k, bass.ts(qx, 512)],
                        start=(k == 0), stop=(k == N_KT - 1),
                    )
                nc.vector.tensor_copy(QTb[:, bass.ts(qx, 512)], qp)

            def emit_extras(qt, ks):
                # deferred work, at most one small piece per chunk, placed
                # after scores(n+1) in the PE queue. O-projection and KQ
                # leftovers go on the bank pair that is off duty for qt.
                if qt == 0:
                    if ks < 2:
                        emit_vblock(po_pool, 0, ks + 2)
                    elif ks < 6:
                        emit_vblock(tr_pool, 1, ks - 2)
                    elif ks < 10:
                        emit_vblock(po_pool, 2, ks - 6)
                    elif ks < 14:
                        emit_vblock(tr_pool, 3, ks - 10)
                elif qt == 1:
                    if ks == 2:
                        emit_qtproj(2, a0_pool)
                    elif ks == 4:
                        emit_qtproj(3, a1_pool)
                    elif ks in (6, 8, 10, 12):
                        emit_oproj_piece(0, (ks - 6) // 2,
                                         a0_pool if ks % 4 == 2 else a1_pool)
                else:
                    if ks in (3, 5, 7, 9):
                        mi = (ks - 3) // 2
                        if qt == 2:
                            emit_oproj_piece(1, mi, po_pool if mi % 2 else tr_pool)
                        else:
                            emit_oproj_piece(2, mi, a0_pool if mi % 2 else a1_pool)

            s_cur = emit_scores(0)
            emit_vblock(po_pool, 0, 0)
            emit_vblock(po_pool, 0, 1)
            for qt in range(N_QT):
                qs = bass.ts(qt, 512)
                lo, hi = (a0_pool, a1_pool) if qt % 2 == 0 else (po_pool, tr_pool)
                av0 = lo.tile([128, 512], F32, tag="b", name="av0")
                av1 = hi.tile([128, 512], F32, tag="b", name="av1")
                for ks in range(N_ST):
                    n = qt * N_ST + ks
                    s_next = emit_scores(n + 1) if n + 1 < N_QT * N_ST else None
                    emit_extras(qt, ks)
                    pt = pt_pool.tile([128, 2, 512], BF16, tag="pt")
                    nc.scalar.activation(
                        out=pt, in_=s_cur, func=mybir.ActivationFunctionType.Exp,
                        scale=SCALE,
                    )
                    s_cur = s_next
                    # M=65: rows 0:64 accumulate V^T @ PT, row 64 (ones
                    # column) accumulates the softmax denominators
                    nc.tensor.matmul(
                        av0[0:HD + 1, :], V_sb[:, ks, 0, :], pt[:, 0, :],
                        start=(ks == 0), stop=(ks == N_ST - 1),
                    )
                    nc.tensor.matmul(
                        av1[0:HD + 1, :], V_sb[:, ks, 1, :], pt[:, 1, :],
                        start=(ks == 0), stop=(ks == N_ST - 1),
                    )
                if qt == N_QT - 1:
                    # keep the PE warm through the final normalize so the
                    # tail O-projection runs at speed
                    for i in range(N_TAIL_FILL):
                        fl = ps_pool.tile([128, 2, 512], F32, tag="ps", name="flt")
                        nc.tensor.matmul(
                            fl[:, 0, :], dummy[:, 0:128], dummy, start=True, stop=True,
                        )
                # normalize: avc + s_row staged first (frees the av banks),
                # then reciprocal -> gpsimd broadcast -> multiply into Z
                avc_sb = norm_pool.tile([64, 2, 512], F32, tag="avc")
                s_row = norm_pool.tile([1, 2, 512], F32, tag="s_row")
                nc.vector.tensor_copy(avc_sb[:, 0, :], av0[0:64, :])
                nc.vector.tensor_copy(s_row[0:1, 0, :], av0[64:65, :])
                nc.vector.tensor_copy(avc_sb[:, 1, :], av1[0:64, :])
                nc.vector.tensor_copy(s_row[0:1, 1, :], av1[64:65, :])
                r0_sb = norm_pool.tile([1, 2, 512], F32, tag="r0")
                nc.vector.reciprocal_approx_fast(
                    out=r0_sb[0:1, 0, :], in_=s_row[0:1, 0, :]
                )
                nc.vector.reciprocal_approx_fast(
                    out=r0_sb[0:1, 1, :], in_=s_row[0:1, 1, :]
                )
                rb_sb = norm_pool.tile([64, 2, 512], F32, tag="rb")
                nc.gpsimd.partition_broadcast(
                    out_ap=rb_sb[0:64, 0, :], in_ap=r0_sb[0:1, 0, :]
                )
                nc.gpsimd.partition_broadcast(
                    out_ap=rb_sb[0:64, 1, :], in_ap=r0_sb[0:1, 1, :]
                )
                nc.vector.tensor_mul(
                    Z_sb[0:64, qs], avc_sb[:, 0, :], rb_sb[0:64, 0, :]
                )
                nc.vector.tensor_mul(
                    Z_sb[64:128, qs], avc_sb[:, 1, :], rb_sb[0:64, 1, :]
                )
            for mi in range(4):
                emit_oproj_piece(N_QT - 1, mi, a0_pool if mi % 2 else a1_pool)

    nc.compile()
    return nc


_NC = None


def _get_nc():
    global _NC
    if _NC is None:
        _NC = build_nc()
    return _NC


def _tiled(a):
    """[512, N] -> [128, 4, N] bf16 with contiguous per-partition rows."""
    n = a.shape[1]
    return np.ascontiguousarray(
        a.reshape(N_KT, 128, n).transpose(1, 0, 2)
    ).astype(ml_dtypes.bfloat16)


def make_in_maps(x, w_qkv, w_o):
    x = np.asarray(x, dtype=np.float32)
    w_qkv = np.asarray(w_qkv, dtype=np.float32)
    w_o = np.asarray(w_o, dtype=np.float32)
    in_maps = []
    xTs = [_tiled(np.ascontiguousarray(x[b].T)) for b in range(B)]
    for c in range(8):
        b, g = c // 4, c % 4
        cols = slice(2 * g * HD, (2 * g + 2) * HD)
        in_maps.append({
            "xTb": xTs[b],
            "wqb": _tiled(w_qkv[:, :EMBED][:, cols]),
            "wkb": _tiled(w_qkv[:, EMBED:2 * EMBED][:, cols]),
            "wvb": _tiled(w_qkv[:, 2 * EMBED:][:, cols]),
            "wo": np.ascontiguousarray(w_o[cols, :]),
        })
    return in_maps


def combine(results, b_o):
    partials = np.stack(
        [np.asarray(r["out"]).astype(np.float32) for r in results]
    )  # [8, S, EMBED]
    out = partials.reshape(B, 4, S, EMBED).sum(axis=1)
    return (out + np.asarray(b_o, dtype=np.float32)).astype(np.float32)


def kernel(x, w_qkv, w_o, b_o):
    nc = _get_nc()
    res = run_bass_kernel_spmd(nc, make_in_maps(x, w_qkv, w_o), core_ids=list(range(8)))
    return combine(res.results, b_o)


# revision 6
# speedup vs baseline: 1.1084x; 1.1084x over previous
"""Trainium2 Bass kernel for nn_MultiHeadAttention (x:[2,2048,512], 8 heads, d=64).

Sharding: 8 cores = 2 batches x 4 head-pairs. Each core computes the QKV
projection for its 2 heads, the attention, and a partial (row-split) O
projection. Host sums the 4 partials per batch and adds the output bias.

All matmul inputs are bf16 (validated ~6e-3 max rel err vs the 2e-2 gate);
accumulation is always fp32 in PSUM. Host pre-arranges every input so each
DMA descriptor row is contiguous. Partial outputs return as bf16.

Per-core schedule (single Act-engine EXP stream is the bound, ~69us):
  t0      DMA in two concurrent waves (per-descriptor bw is ~100GB/s,
          aggregate ~270GB/s): wave 1 = xT k0,k1 + wk,wq on sync+gpsimd;
          a gpsimd guard reads k0 and writes into k2's region (real WAW
          dep) gating wave 2 = k2,k3 + wv,wo. PE warm-up fillers ramp the
          p-state through the wait.
  phase A k-major KQ projection accumulating in psum per arriving k-tile:
          KT qt0-3 (ps pool banks) + QT qt0/qt1 (av banks); the last k
          row emits KT0/QT0 first so their casts gate scores(0) only
  V proj  bf16: 4 seq-blocks share one PSUM bank (sequential per-region
          accumulation groups on the in-order PE); 2 blocks up front,
          the rest threaded one block per chunk
  chunks  64 chunks (qt,ks), per-chunk PE order: scores(n+1) FIRST, then
          at most ~one matmul of deferred work, then AV(n). EXP on Act
          (psum fp32 -> sbuf bf16, scale fused); AV with M=65 ones-column
          trick. AV banks ALTERNATE per qt parity (a0/a1 <-> po/tr) so
          the next qt's AV opens banks freed chunks earlier - no
          head-of-line stall on the in-order PE queue at qt boundaries.
  norm    per qt: avc + s_row staged (frees av banks), reciprocal, gpsimd
          partition broadcast, DVE multiply -> Z. O-projection of qt-1
          threaded as N=256 half-pieces, one per chunk, on the off-duty
          banks; one batched out-DMA per qt.
  tail    ps-pool fillers keep the PE warm; s_row + reciprocals reordered
          first; the broadcast runs as a K=1 PE matmul (ones x r0) into a
          ps tile; O-projection N=512 x4 alternating a0/a1; out-DMA in
          two halves so transfer overlaps the last pieces.
"""

import sys

import numpy as np

for _p in ("/opt/trn_rl_repo",):
    if _p not in sys.path:
        sys.path.insert(0, _p)

import ml_dtypes  # noqa: E402

import concourse.bass as bass  # noqa: E402
import concourse.tile as tile  # noqa: E402
from concourse import bacc, mybir  # noqa: E402
from concourse.bass_utils import run_bass_kernel_spmd  # noqa: E402

EMBED = 512
NH = 8
HD = 64
S = 2048
B = 2
SCALE = HD ** -0.5
F32 = mybir.dt.float32
F32R = mybir.dt.float32r
BF16 = mybir.dt.bfloat16

N_KT = EMBED // 128   # 4 contraction k-tiles for the projections
N_QT = S // 512       # 4 q column tiles
N_ST = S // 128       # 16 seq tiles of 128

N_FILL = 10           # PE warm-up matmuls during the DMA phase
N_TAIL_FILL = 10      # PE keep-warm matmuls through the tail normalize


def build_nc():
    nc = bacc.Bacc("TRN2", target_bir_lowering=False, debug=False)

    xTb_d = nc.dram_tensor("xTb", [128, N_KT, S], BF16, kind="ExternalInput").ap()
    wqb_d = nc.dram_tensor("wqb", [128, N_KT, 128], BF16, kind="ExternalInput").ap()
    wkb_d = nc.dram_tensor("wkb", [128, N_KT, 128], BF16, kind="ExternalInput").ap()
    wvb_d = nc.dram_tensor("wvb", [128, N_KT, 128], BF16, kind="ExternalInput").ap()
    wo_d = nc.dram_tensor("wo", [128, EMBED], F32R, kind="ExternalInput").ap()
    out_d = nc.dram_tensor("out", [S, EMBED], BF16, kind="ExternalOutput").ap()
    out_r = out_d.rearrange("(m p) e -> p m e", p=128)

    with tile.TileContext(nc) as tc:
        with (
            tc.tile_pool(name="persist", bufs=1) as persist,
            tc.tile_pool(name="pt_pool", bufs=6) as pt_pool,
            tc.tile_pool(name="norm", bufs=2) as norm_pool,
            tc.tile_pool(name="ostage", bufs=2) as ostage,
            tc.tile_pool(name="ps", bufs=2, space="PSUM") as ps_pool,
            tc.tile_pool(name="ps_a0", bufs=1, space="PSUM") as a0_pool,
            tc.tile_pool(name="ps_a1", bufs=1, space="PSUM") as a1_pool,
            tc.tile_pool(name="ps_po", bufs=1, space="PSUM") as po_pool,
            tc.tile_pool(name="ps_tr", bufs=1, space="PSUM") as tr_pool,
        ):
            # ---- DMA wave 1: k0 + k1 + wk + wq ----
            wkb_sb = persist.tile([128, N_KT, 128], BF16)
            wqb_sb = persist.tile([128, N_KT, 128], BF16)
            wvb_sb = persist.tile([128, N_KT, 128], BF16)
            wo_sb = persist.tile([128, EMBED], F32R)
            xTb_sb = persist.tile([128, N_KT, S], BF16)
            nc.sync.dma_start(out=xTb_sb[:, 0, :], in_=xTb_d[:, 0, :])
            nc.sync.dma_start(out=wkb_sb, in_=wkb_d)
            nc.sync.dma_start(out=wqb_sb, in_=wqb_d)

            dummy = persist.tile([128, 512], BF16)
            nc.gpsimd.memset(dummy, 0.0)
            nc.gpsimd.dma_start(out=xTb_sb[:, 1, :], in_=xTb_d[:, 1, :])
            # guard: read k0, write 2 elems into k2's region -> the k2 DMA
            # (full overwrite) gets a WAW dep, serializing wave 2 after k0
            nc.gpsimd.tensor_scalar_add(
                out=xTb_sb[0:1, 2, 0:2], in0=xTb_sb[0:1, 0, 0:2], scalar1=0.0,
            )
            nc.gpsimd.dma_start(out=xTb_sb[:, 2, :], in_=xTb_d[:, 2, :])
            nc.gpsimd.dma_start(out=xTb_sb[:, 3, :], in_=xTb_d[:, 3, :])
            nc.gpsimd.dma_start(out=wvb_sb, in_=wvb_d)
            nc.gpsimd.dma_start(out=wo_sb, in_=wo_d)
            # V with a baked all-ones 65th column: the M=65 AV matmuls then
            # produce the softmax denominators in psum row 64 for free
            V_sb = persist.tile([128, N_ST, 2, HD + 1], BF16)
            nc.gpsimd.memset(V_sb, 1.0)
            ones_sb = persist.tile([1, HD], BF16)
            nc.gpsimd.memset(ones_sb, 1.0)

            # ---- PE warm-up fillers (p-state ramp through the DMA wait) ----
            fill_pools = (a0_pool, a1_pool, po_pool, tr_pool)
            for i in range(N_FILL):
                fl = fill_pools[i % 4].tile([128, 512], F32, tag="b", name="fl")
                nc.tensor.matmul(fl, dummy[:, 0:128], dummy, start=True, stop=True)

            # ---- phase A: k-major KQ projection. The last k row emits
            # KT qt0 / QT qt0 first so their casts gate scores(0) only ----
            KTps = [ps_pool.tile([128, 2, 512], F32, tag="ps", name="KTps") for _ in range(2)]
            QT0ps = a0_pool.tile([128, 512], F32, tag="b", name="QT0ps")
            QT1ps = a1_pool.tile([128, 512], F32, tag="b", name="QT1ps")

            def kq_mm(j, k):
                st, sp = k == 0, k == N_KT - 1
                if j < 4:
                    nc.tensor.matmul(
                        KTps[j // 2][:, j % 2, :],
                        wkb_sb[:, k, :], xTb_sb[:, k, bass.ts(j, 512)],
                        start=st, stop=sp,
                    )
                else:
                    nc.tensor.matmul(
                        QT0ps if j == 4 else QT1ps, wqb_sb[:, k, :],
                        xTb_sb[:, k, bass.ts(j - 4, 512)], start=st, stop=sp,
                    )

            for k in range(N_KT - 1):
                for j in range(6):
                    kq_mm(j, k)
            for j in (0, 4, 1, 5, 2, 3):  # KT0, QT0 first on the last k row
                kq_mm(j, N_KT - 1)
            KTb = persist.tile([128, S], BF16)
            QTb = persist.tile([128, S], BF16)
            nc.vector.tensor_copy(KTb[:, 0:512], KTps[0][:, 0, :])
            nc.vector.tensor_copy(QTb[:, 0:512], QT0ps)
            nc.vector.tensor_copy(QTb[:, 512:1024], QT1ps)
            nc.vector.tensor_copy(KTb[:, 512:1024], KTps[0][:, 1, :])
            nc.vector.tensor_copy(KTb[:, 1024:1536], KTps[1][:, 0, :])
            nc.vector.tensor_copy(KTb[:, 1536:2048], KTps[1][:, 1, :])

            # ---- V projection: 4 seq-blocks [128,128] side by side in one
            # psum bank; per-region accumulation groups run sequentially on
            # the in-order PE so plain start/stop per block is safe ----
            vg_state = {}

            def emit_vblock(pool, g, b):
                if b == 0:
                    vg_state[g] = pool.tile([128, 512], F32, tag="b", name="vg")
                vg = vg_state[g]
                stq = 4 * g + b
                for k in range(N_KT):
                    nc.tensor.matmul(
                        vg[:, bass.ts(b, 128)],
                        xTb_sb[:, k, bass.ts(stq, 128)], wvb_sb[:, k, :],
                        start=(k == 0), stop=(k == N_KT - 1),
                    )
                nc.vector.tensor_copy(
                    V_sb[:, stq, :, 0:HD],
                    vg.rearrange("p (b h d) -> p b h d", b=4, h=2)[:, b],
                )

            # ---- attention chunk stream ----
            Z_sb = persist.tile([128, S], F32R)  # normalized attn out^T, 2 heads
            ost_state = {}
            qp_state = {}

            def emit_scores(n):
                qt, ks = n // N_ST, n % N_ST
                s = ps_pool.tile([128, 2, 512], F32, tag="ps", name="s")
                kk = bass.ts(ks, 128)
                qs = bass.ts(qt, 512)
                nc.tensor.matmul(
                    s[:, 0, :], KTb[0:64, kk], QTb[0:64, qs], start=True, stop=True,
                )
                nc.tensor.matmul(
                    s[:, 1, :], KTb[64:128, kk], QTb[64:128, qs], start=True, stop=True,
                )
                return s

            def emit_oproj_half(src_qt, hi, pool):
                # one N=256 half m-block of the O-projection for src_qt;
                # 8 halves land in one ostage tile, one DMA per qt
                m, cs = hi // 2, bass.ts(hi % 2, 256)
                if hi == 0:
                    ost_state[src_qt] = ostage.tile(
                        [128, 4, 512], BF16, tag="ot", name="ost"
                    )
                po = pool.tile([128, 512], F32, tag="b", name="po")
                nc.tensor.matmul(
                    po[:, 0:256], Z_sb[:, bass.ts(4 * src_qt + m, 128)], wo_sb[:, cs],
                    start=True, stop=True,
                )
                nc.vector.tensor_copy(ost_state[src_qt][:, m, cs], po[:, 0:256])
                if hi == 7:
                    nc.sync.dma_start(
                        out=out_r[:, 4 * src_qt:4 * src_qt + 4, :],
                        in_=ost_state[src_qt],
                    )

            def emit_qtproj_mm(qx, k, pool):
                if k == 0:
                    qp_state[qx] = pool.tile([128, 512], F32, tag="b", name="qp")
                nc.tensor.matmul(
                    qp_state[qx], wqb_sb[:, k, :], xTb_sb[:, k, bass.ts(qx, 512)],
                    start=(k == 0), stop=(k == N_KT - 1),
                )
                if k == N_KT - 1:
                    nc.vector.tensor_copy(QTb[:, bass.ts(qx, 512)], qp_state[qx])

            def emit_extras(qt, ks):
                # deferred work, at most ~one matmul per chunk, placed after
                # scores(n+1) in the PE queue, on banks off duty for qt
                if qt == 0:
                    if ks < 2:
                        emit_vblock(po_pool, 0, ks + 2)
                    elif ks < 6:
                        emit_vblock(tr_pool, 1, ks - 2)
                    elif ks < 10:
                        emit_vblock(po_pool, 2, ks - 6)
                    elif ks < 14:
                        emit_vblock(tr_pool, 3, ks - 10)
                elif qt == 1:
                    if 2 <= ks < 6:
                        emit_qtproj_mm(2, ks - 2, a0_pool)
                    elif 7 <= ks < 11:
                        emit_qtproj_mm(3, ks - 7, a1_pool)
                    elif 11 <= ks < 15:
                        emit_oproj_half(0, ks - 11, a0_pool)
                elif qt == 2:
                    if ks < 4:
                        emit_oproj_half(0, 4 + ks, po_pool if ks % 2 else tr_pool)
                    elif ks < 12:
                        emit_oproj_half(1, ks - 4, po_pool if ks % 2 else tr_pool)
                else:
                    if 3 <= ks < 11:
                        emit_oproj_half(2, ks - 3, a0_pool if ks % 2 else a1_pool)

            s_cur = emit_scores(0)
            emit_vblock(po_pool, 0, 0)
            emit_vblock(po_pool, 0, 1)
            for qt in range(N_QT):
                qs = bass.ts(qt, 512)
                lo, hi = (a0_pool, a1_pool) if qt % 2 == 0 else (po_pool, tr_pool)
                av0 = lo.tile([128, 512], F32, tag="b", name="av0")
                av1 = hi.tile([128, 512], F32, tag="b", name="av1")
                for ks in range(N_ST):
                    n = qt * N_ST + ks
                    s_next = emit_scores(n + 1) if n + 1 < N_QT * N_ST else None
                    emit_extras(qt, ks)
                    pt = pt_pool.tile([128, 2, 512], BF16, tag="pt")
                    nc.scalar.activation(
                        out=pt, in_=s_cur, func=mybir.ActivationFunctionType.Exp,
                        scale=SCALE,
                    )
                    s_cur = s_next
                    # M=65: rows 0:64 accumulate V^T @ PT, row 64 (ones
                    # column) accumulates the softmax denominators
                    nc.tensor.matmul(
                        av0[0:HD + 1, :], V_sb[:, ks, 0, :], pt[:, 0, :],
                        start=(ks == 0), stop=(ks == N_ST - 1),
                    )
                    nc.tensor.matmul(
                        av1[0:HD + 1, :], V_sb[:, ks, 1, :], pt[:, 1, :],
                        start=(ks == 0), stop=(ks == N_ST - 1),
                    )
                tail = qt == N_QT - 1
                if tail:
                    # keep the PE warm through the final normalize so the
                    # tail broadcast + O-projection run at speed
                    for i in range(N_TAIL_FILL):
                        fl = ps_pool.tile([128, 2, 512], F32, tag="ps", name="flt")
                        nc.tensor.matmul(
                            fl[:, 0, :], dummy[:, 0:128], dummy, start=True, stop=True,
                        )
                # normalize. Steady qts: avc + s_row first (frees av banks
                # for reuse 2 qts later). Tail: s_row + recips first (they
                # gate the broadcast), broadcast on the PE (K=1 matmul).
                avc_sb = norm_pool.tile([64, 2, 512], F32, tag="avc")
                s_row = norm_pool.tile([1, 2, 512], F32, tag="s_row")
                r0_sb = norm_pool.tile([1, 2, 512], F32, tag="r0")
                if not tail:
                    nc.vector.tensor_copy(avc_sb[:, 0, :], av0[0:64, :])
                    nc.vector.tensor_copy(s_row[0:1, 0, :], av0[64:65, :])
                    nc.vector.tensor_copy(avc_sb[:, 1, :], av1[0:64, :])
                    nc.vector.tensor_copy(s_row[0:1, 1, :], av1[64:65, :])
                else:
                    nc.vector.tensor_copy(s_row[0:1, 0, :], av0[64:65, :])
                    nc.vector.tensor_copy(s_row[0:1, 1, :], av1[64:65, :])
                nc.vector.reciprocal_approx_fast(
                    out=r0_sb[0:1, 0, :], in_=s_row[0:1, 0, :]
                )
                nc.vector.reciprocal_approx_fast(
                    out=r0_sb[0:1, 1, :], in_=s_row[0:1, 1, :]
                )
                if tail:
                    r0b = norm_pool.tile([1, 2, 512], BF16, tag="r0b")
                    nc.vector.tensor_copy(r0b, r0_sb)
                    rbp = [
                        ps_pool.tile([128, 2, 512], F32, tag="ps", name="rbp")
                        for _ in range(2)
                    ]
                    for h in range(2):
                        nc.tensor.matmul(
                            rbp[h][0:64, 0, :], ones_sb, r0b[0:1, h, :],
                            start=True, stop=True,
                        )
                    nc.vector.tensor_copy(avc_sb[:, 0, :], av0[0:64, :])
                    nc.vector.tensor_copy(avc_sb[:, 1, :], av1[0:64, :])
                    nc.vector.tensor_mul(
                        Z_sb[0:64, qs], avc_sb[:, 0, :], rbp[0][0:64, 0, :]
                    )
                    nc.vector.tensor_mul(
                        Z_sb[64:128, qs], avc_sb[:, 1, :], rbp[1][0:64, 0, :]
                    )
                else:
                    rb_sb = norm_pool.tile([64, 2, 512], F32, tag="rb")
                    nc.gpsimd.partition_broadcast(
                        out_ap=rb_sb[0:64, 0, :], in_ap=r0_sb[0:1, 0, :]
                    )
                    nc.gpsimd.partition_broadcast(
                        out_ap=rb_sb[0:64, 1, :], in_ap=r0_sb[0:1, 1, :]
                    )
                    nc.vector.tensor_mul(
                        Z_sb[0:64, qs], avc_sb[:, 0, :], rb_sb[0:64, 0, :]
                    )
                    nc.vector.tensor_mul(
                        Z_sb[64:128, qs], avc_sb[:, 1, :], rb_sb[0:64, 1, :]
                    )
            # tail O-projection: N=512 per m-block, alternating a0/a1, with
            # the out-DMA split in two halves so transfer overlaps compute
            ost = ostage.tile([128, 4, 512], BF16, tag="ot", name="ost_t")
            for mi in range(4):
                po = (a0_pool if mi % 2 else a1_pool).tile(
                    [128, 512], F32, tag="b", name="po_t"
                )
                nc.tensor.matmul(
                    po, Z_sb[:, bass.ts(12 + mi, 128)], wo_sb, start=True, stop=True,
                )
                nc.vector.tensor_copy(ost[:, mi, :], po)
                if mi % 2 == 1:
                    nc.sync.dma_start(
                        out=out_r[:, 12 + mi - 1:12 + mi + 1, :],
                        in_=ost[:, mi - 1:mi + 1, :],
                    )

    nc.compile()
    return nc


_NC = None


def _get_nc():
    global _NC
    if _NC is None:
        _NC = build_nc()
    return _NC


def _tiled(a):
    """[512, N] -> [128, 4, N] bf16 with contiguous per-partition rows."""
    n = a.shape[1]
    return np.ascontiguousarray(
        a.reshape(N_KT, 128, n).transpose(1, 0, 2)
    ).astype(ml_dtypes.bfloat16)


def make_in_maps(x, w_qkv, w_o):
    x = np.asarray(x, dtype=np.float32)
    w_qkv = np.asarray(w_qkv, dtype=np.float32)
    w_o = np.asarray(w_o, dtype=np.float32)
    in_maps = []
    xTs = [_tiled(np.ascontiguousarray(x[b].T)) for b in range(B)]
    for c in range(8):
        b, g = c // 4, c % 4
        cols = slice(2 * g * HD, (2 * g + 2) * HD)
        in_maps.append({
            "xTb": xTs[b],
            "wqb": _tiled(w_qkv[:, :EMBED][:, cols]),
            "wkb": _tiled(w_qkv[:, EMBED:2 * EMBED][:, cols]),
            "wvb": _tiled(w_qkv[:, 2 * EMBED:][:, cols]),
            "wo": np.ascontiguousarray(w_o[cols, :]),
        })
    return in_maps


def combine(results, b_o):
    partials = np.stack(
        [np.asarray(r["out"]).astype(np.float32) for r in results]
    )  # [8, S, EMBED]
    out = partials.reshape(B, 4, S, EMBED).sum(axis=1)
    return (out + np.asarray(b_o, dtype=np.float32)).astype(np.float32)


def kernel(x, w_qkv, w_o, b_o):
    nc = _get_nc()
    res = run_bass_kernel_spmd(nc, make_in_maps(x, w_qkv, w_o), core_ids=list(range(8)))
    return combine(res.results, b_o)
